# revision 1
# baseline (speedup 1.0000x reference)
"""Causal self-attention (B=4, T=4096, D=H=1024, fp32) on 8 Trainium2 cores.

Sharding: 2 cores per batch element. Within a batch, the 32 query tiles of
128 rows are interleaved between the 2 cores (core `pair` p takes global
q-tiles p, p+2, p+4, ...), which balances the causal-attention work exactly.
Both cores run the same module; causality is enforced by per-pair mask DATA
on the last two kv-blocks of each q-tile.

Layout: the host stages x pre-transposed (xT, [D, T]) so no PE transposes
are needed for the Q/K/V projections. Attention scores are computed
TRANSPOSED, ST[k, q] = K @ Q^T, so that the exp'd probabilities are already
in the [k, q] layout that the P@V matmul needs as its stationary operand —
no P transposes either. Per-query softmax denominators are accumulated with
a ones-column matmul (N=1) interleaved with the PV matmuls.

Numerics: bf16 matmuls with fp32 PSUM accumulation, softmax without
max-subtraction (scores ~N(0,1) after the 1/32 scale), exp on ScalarE in
fp32 -> bf16 probabilities, final normalization in fp32.
"""

import numpy as np

B, T, D, H = 4, 4096, 1024, 1024
P = 128
NCORES = 8


DEFAULT_CFG = dict(
    phases="AB",
    pa_xt_bufs=2,
    pa_psk_bufs=2, pa_psv_bufs=2,
    pb_xt_bufs=2, pb_qt_bufs=1,
    pb_p_bufs=4, pb_ob_bufs=2,
    pb_q_bufs=2, pb_st_bufs=3, pb_po_bufs=2,
    s_ahead=2,
    sc_desc=True,          # process superchunks in descending order
    warm_mms=0,           # HAM warm-up matmuls before the first projection
)


def _emit(ctx, tc, xqT, xkvT, wq, wk, wv, maskt, ones, outp, T_kv, n_qt, cfg):
    import concourse.mybir as mybir

    nc = tc.nc
    f32 = mybir.dt.float32
    bf16 = mybir.dt.bfloat16
    Copy = mybir.ActivationFunctionType.Copy
    Exp = mybir.ActivationFunctionType.Exp
    SCALE = 1.0 / 32.0  # 1/sqrt(H)

    NKC_A = T_kv // 512   # phase-A 512-row projection chunks
    NSC = n_qt // 4       # 512-row query superchunks
    if "A" not in cfg["phases"]:
        NKC_A = 0
    if "B" not in cfg["phases"]:
        NSC = 0

    const = ctx.enter_context(tc.tile_pool(name="const", bufs=1))
    persist = ctx.enter_context(tc.tile_pool(name="persist", bufs=1))

    # maskt[:, 0:128] masks a tile's second-to-last kv block, maskt[:,128:256]
    # its last kv block ([k, q] layout). pair 0: [triu, 0]; pair 1: [1, triu].
    mask_sb = const.tile([P, 2 * P], bf16, tag="mask")
    nc.sync.dma_start(out=mask_sb, in_=maskt)
    ones_sb = const.tile([P, 1], bf16, tag="ones")
    nc.sync.dma_start(out=ones_sb, in_=ones)

    # K^T laid out [h%128, h//128, t]; V laid out [t%128, t//128, h]
    KT = persist.tile([P, 8, T_kv], bf16, tag="KT")
    V = persist.tile([P, T_kv // P, 1024], bf16, tag="V")
    # wq lives at top level so phase B needn't wait on its DMA; issued inside
    # phase A (after the first chunk's loads) so it doesn't delay wk/xt0.
    wq_sb = persist.tile([P, 8, 1024], bf16, tag="wq")
    # prefetch of the first superchunk's x^T slice, same reasoning
    xtq0 = persist.tile([P, 8, 512], bf16, tag="xtq0")

    def load_weight(wdram, wsb, eng=None):
        # DRAM [1024,1024] bf16 -> SBUF [128, 8, 1024] (d = dc*128 + p)
        eng = eng or nc.sync
        for dc in range(8):
            eng.dma_start(out=wsb[:, dc, :], in_=wdram[dc * P:(dc + 1) * P, :])

    def load_xt(xt, xsrc, c0, width, eng=None, split=False):
        # xt[:, dc, :] = xT[dc*128:(dc+1)*128, c0:c0+width]
        for dc in range(8):
            e = eng or ((nc.sync if dc % 2 else nc.gpsimd) if split else nc.gpsimd)
            e.dma_start(out=xt[:, dc, :],
                        in_=xsrc[dc * P:(dc + 1) * P, c0:c0 + width])

    from contextlib import ExitStack as _ES

    # ---------------- Phase A: K/V projection over all kv rows ----------------
    with _ES() as pa:
        wpool = pa.enter_context(tc.tile_pool(name="pa_w", bufs=1))
        xtpool = pa.enter_context(tc.tile_pool(name="pa_xt", bufs=cfg["pa_xt_bufs"]))
        psA_k = pa.enter_context(
            tc.tile_pool(name="pa_psk", bufs=cfg["pa_psk_bufs"], space="PSUM"))
        psA_v = pa.enter_context(
            tc.tile_pool(name="pa_psv", bufs=cfg["pa_psv_bufs"], space="PSUM"))
        wk_sb = wpool.tile([P, 8, 1024], bf16, tag="wk")
        wv_sb = wpool.tile([P, 8, 1024], bf16, tag="wv")
        load_weight(wk, wk_sb)

        # HAM warm-up: keep the PE busy on throwaway matmuls while the first
        # chunk's DMAs land, so the real stream starts at the 2.4 GHz clock.
        if cfg["warm_mms"]:
            warm_p = pa.enter_context(
                tc.tile_pool(name="pa_warm", bufs=1, space="PSUM"))
            warm_ps = warm_p.tile([P, P], f32, tag="warm")
            for i in range(cfg["warm_mms"]):
                nc.tensor.matmul(warm_ps, lhsT=mask_sb[:, 0:P],
                                 rhs=mask_sb[:, 0:P],
                                 start=(i == 0), stop=(i == cfg["warm_mms"] - 1))

        sc0 = (NSC - 1) if cfg["sc_desc"] else 0
        for c in range(NKC_A):
            t0 = c * 512
            xt = xtpool.tile([P, 8, 512], bf16, tag="xt")
            load_xt(xt, xkvT, t0, 512)
            if c == 0:
                # stream the rest of the initial working set behind wk+xt0,
                # on the otherwise-idle Activation DMA queue
                load_weight(wv, wv_sb, eng=nc.scalar)
                load_weight(wq, wq_sb, eng=nc.scalar)
                load_xt(xtq0, xqT, sc0 * 512, 512, eng=nc.sync)
            # K^T_[h, t0:t0+512] = Wk^T @ x^T
            for hc in range(8):
                kp = psA_k.tile([P, 512], f32, tag="kp")
                for dc in range(8):
                    nc.tensor.matmul(
                        kp, lhsT=wk_sb[:, dc, hc * P:(hc + 1) * P],
                        rhs=xt[:, dc, :], start=(dc == 0), stop=(dc == 7))
                nc.vector.tensor_copy(out=KT[:, hc, t0:t0 + 512], in_=kp)
            # V_[t0+i*128, :] = x @ Wv
            for i in range(4):
                vp = psA_v.tile([P, 1024], f32, tag="vp")
                for dc in range(8):
                    for nb in range(2):
                        nc.tensor.matmul(
                            vp[:, nb * 512:(nb + 1) * 512],
                            lhsT=xt[:, dc, i * P:(i + 1) * P],
                            rhs=wv_sb[:, dc, nb * 512:(nb + 1) * 512],
                            start=(dc == 0), stop=(dc == 7))
                nc.vector.tensor_copy(out=V[:, t0 // P + i, :], in_=vp)

    # ---------------- Phase B: Q projection + attention ----------------
    with _ES() as pb_es:
        ec = pb_es.enter_context
        xtq_p = ec(tc.tile_pool(name="pb_xt", bufs=cfg["pb_xt_bufs"]))
        qt_p = ec(tc.tile_pool(name="pb_qt", bufs=cfg["pb_qt_bufs"]))
        pb_p = ec(tc.tile_pool(name="pb_p", bufs=cfg["pb_p_bufs"]))
        sums_p = ec(tc.tile_pool(name="pb_sums", bufs=4))
        ob_p = ec(tc.tile_pool(name="pb_ob", bufs=cfg["pb_ob_bufs"]))
        ps_st = ec(tc.tile_pool(name="pb_st", bufs=cfg["pb_st_bufs"], space="PSUM"))
        ps_o = ec(tc.tile_pool(name="pb_po", bufs=cfg["pb_po_bufs"], space="PSUM"))
        ps_sum = ec(tc.tile_pool(name="pb_psum_s", bufs=1, space="PSUM"))

        sc_order = list(range(NSC))
        if cfg["sc_desc"]:
            sc_order = sc_order[::-1]

        for si_sc, sc in enumerate(sc_order):
            # Q^T for this superchunk: [h%128, h//128, 512 local q]
            if si_sc == 0:
                xtq = xtq0
            else:
                xtq = xtq_p.tile([P, 8, 512], bf16, tag="xtq")
                load_xt(xtq, xqT, sc * 512, 512)
            qt = qt_p.tile([P, 8, 512], bf16, tag="qt")
            for hc in range(8):
                for qh in range(2):
                    qp = ps_st.tile([P, 256], f32, tag="sp", name="qp")
                    for dc in range(8):
                        nc.tensor.matmul(
                            qp, lhsT=wq_sb[:, dc, hc * P:(hc + 1) * P],
                            rhs=xtq[:, dc, qh * 256:(qh + 1) * 256],
                            start=(dc == 0), stop=(dc == 7))
                    nc.vector.tensor_copy(
                        out=qt[:, hc, qh * 256:(qh + 1) * 256], in_=qp)

            for m in range(2):
                jA = sc * 4 + 2 * m
                jB = jA + 1
                nbA, nbB = 2 * jA + 2, 2 * jB + 2  # kv blocks incl. 2 masked
                qoff = 256 * m
                # steps: (kb, width, q offset, [(tile_key, ST col offset)])
                steps = []
                for kb in range(nbA):
                    steps.append((kb, 256, qoff, [("A", 0), ("B", P)]))
                for kb in range(nbA, nbB):
                    steps.append((kb, 128, qoff + P, [("B", 0)]))
                nst = len(steps)

                op = {"A": ps_o.tile([P, 1024], f32, tag="op", name="opA"),
                      "B": ps_o.tile([P, 1024], f32, tag="op", name="opB")}
                sums = ps_sum.tile([P, 2], f32, tag="sums")
                scol = {"A": 0, "B": 1}
                lastb = {"A": nbA - 1, "B": nbB - 1}

                def s_mm(si):
                    kb, w, qo, _tiles = steps[si]
                    sp = ps_st.tile([P, w], f32, tag="sp")
                    for hc in range(8):
                        nc.tensor.matmul(
                            sp, lhsT=KT[:, hc, kb * P:(kb + 1) * P],
                            rhs=qt[:, hc, qo:qo + w],
                            start=(hc == 0), stop=(hc == 7))
                    return sp

                def softmax(si, sp):
                    kb, w, qo, tiles = steps[si]
                    pb = pb_p.tile([P, w], bf16, tag="pb")
                    nc.scalar.activation(out=pb, in_=sp, func=Exp, scale=SCALE)
                    for tk, off in tiles:
                        d = kb - (lastb[tk] - 1)   # 0: 2nd-to-last, 1: last
                        if d >= 0:
                            nc.vector.tensor_mul(
                                pb[:, off:off + P], pb[:, off:off + P],
                                mask_sb[:, d * P:(d + 1) * P])
                    return pb

                def pv(si, pb):
                    kb, w, qo, tiles = steps[si]
                    for tk, off in tiles:
                        o = op[tk]
                        first = (kb == 0)
                        last = (kb == lastb[tk])
                        for nb in range(2):
                            nc.tensor.matmul(
                                o[:, nb * 512:(nb + 1) * 512],
                                lhsT=pb[:, off:off + P],
                                rhs=V[:, kb, nb * 512:(nb + 1) * 512],
                                start=first, stop=last)
                        # sums cols A/B share one PSUM bank; start=True zeroes
                        # the WHOLE bank, so only the group's very first sums
                        # matmul (tile A, kb 0) starts, and only the group's
                        # final one (tile B's last) stops.
                        nc.tensor.matmul(
                            sums[:, scol[tk]:scol[tk] + 1],
                            lhsT=pb[:, off:off + P], rhs=ones_sb,
                            start=(first and tk == "A"),
                            stop=(last and tk == "B"),
                            skip_group_check=True)

                ahead = cfg["s_ahead"]
                sps, pbs = {}, {}
                for si in range(min(ahead, nst)):
                    sps[si] = s_mm(si)
                    pbs[si] = softmax(si, sps[si])
                for si in range(nst):
                    pv(si, pbs[si])
                    if si + ahead < nst:
                        sps[si + ahead] = s_mm(si + ahead)
                        pbs[si + ahead] = softmax(si + ahead, sps[si + ahead])

                for tk, j in (("A", jA), ("B", jB)):
                    tot = sums_p.tile([P, 1], f32, tag="tot")
                    nc.vector.tensor_copy(out=tot,
                                          in_=sums[:, scol[tk]:scol[tk] + 1])
                    rec = sums_p.tile([P, 1], f32, tag="rec")
                    nc.vector.reciprocal(out=rec, in_=tot)
                    ob = ob_p.tile([P, 1024], f32, tag="ob")
                    nc.scalar.activation(out=ob, in_=op[tk], func=Copy,
                                         scale=rec)
                    nc.sync.dma_start(out=outp[j * P:(j + 1) * P, :], in_=ob)


def build_module(T_kv=T, n_qt=None, cfg=None):
    from contextlib import ExitStack
    import concourse.tile as tile
    import concourse.mybir as mybir
    from concourse import bacc

    if n_qt is None:
        n_qt = T_kv // 256
    full_cfg = dict(DEFAULT_CFG)
    if cfg:
        full_cfg.update(cfg)
    cfg = full_cfg
    dt = mybir.dt
    nc = bacc.Bacc("TRN2", target_bir_lowering=False, debug=False,
                   num_devices=NCORES)
    xqT = nc.dram_tensor("xqt", [D, n_qt * P], dt.bfloat16, kind="ExternalInput").ap()
    xkvT = nc.dram_tensor("xkvt", [D, T_kv], dt.bfloat16, kind="ExternalInput").ap()
    wq = nc.dram_tensor("wq", [D, H], dt.bfloat16, kind="ExternalInput").ap()
    wk = nc.dram_tensor("wk", [D, H], dt.bfloat16, kind="ExternalInput").ap()
    wv = nc.dram_tensor("wv", [D, H], dt.bfloat16, kind="ExternalInput").ap()
    maskt = nc.dram_tensor("maskt", [P, 2 * P], dt.bfloat16, kind="ExternalInput").ap()
    ones = nc.dram_tensor("ones", [P, 1], dt.bfloat16, kind="ExternalInput").ap()
    outp = nc.dram_tensor("outp", [n_qt * P, H], dt.float32, kind="ExternalOutput").ap()

    with tile.TileContext(nc) as tc:
        with ExitStack() as ctx:
            _emit(ctx, tc, xqT, xkvT, wq, wk, wv, maskt, ones, outp, T_kv,
                  n_qt, cfg)
    nc.compile()
    return nc


def host_inputs(x, Wq, Wk, Wv, T_kv=T, n_qt=None, n_batch=None):
    """Build the per-core input maps for run_bass_kernel_spmd."""
    import ml_dtypes
    bf = ml_dtypes.bfloat16
    if n_qt is None:
        n_qt = T_kv // 256
    if n_batch is None:
        n_batch = x.shape[0]
    triu = np.triu(np.ones((P, P), np.float32))   # mask[k,q] = 1 iff k<=q
    m = [np.concatenate([triu, np.zeros((P, P), np.float32)], 1).astype(bf),
         np.concatenate([np.ones((P, P), np.float32), triu], 1).astype(bf)]
    onesv = np.ones((P, 1), np.float32).astype(bf)

    xb = np.asarray(x, np.float32).astype(bf)
    wqb = np.asarray(Wq, np.float32).astype(bf)
    wkb = np.asarray(Wk, np.float32).astype(bf)
    wvb = np.asarray(Wv, np.float32).astype(bf)
    in_maps = []
    for c in range(NCORES):
        b, pair = (c // 2) % n_batch, c % 2
        xT = np.ascontiguousarray(xb[b].T)            # [D, T]
        qcols = np.concatenate(
            [xT[:, (2 * j + pair) * P:(2 * j + pair + 1) * P]
             for j in range(n_qt)], axis=1)
        in_maps.append({
            "xqt": np.ascontiguousarray(qcols),
            "xkvt": xT,
            "wq": wqb, "wk": wkb, "wv": wvb,
            "maskt": m[pair], "ones": onesv,
        })
    return in_maps


def gather_output(results, T_kv=T, n_qt=None, n_batch=B):
    if n_qt is None:
        n_qt = T_kv // 256
    out = np.empty((n_batch, T_kv, H), np.float32)
    for c in range(2 * n_batch):
        b, pair = c // 2, c % 2
        r = results[c]["outp"]
        for j in range(n_qt):
            out[b, (2 * j + pair) * P:(2 * j + pair + 1) * P, :] = \
                r[j * P:(j + 1) * P, :]
    return out


_NC_CACHE = {}


def kernel(x, Wq, Wk, Wv):
    from concourse.bass_utils import run_bass_kernel_spmd

    x = np.asarray(x, dtype=np.float32)
    Wq = np.asarray(Wq, dtype=np.float32)
    Wk = np.asarray(Wk, dtype=np.float32)
    Wv = np.asarray(Wv, dtype=np.float32)

    if "nc" not in _NC_CACHE:
        _NC_CACHE["nc"] = build_module()
    nc = _NC_CACHE["nc"]

    in_maps = host_inputs(x, Wq, Wk, Wv)
    res = run_bass_kernel_spmd(nc, in_maps, core_ids=list(range(NCORES)))
    return gather_output(res.results)



# revision 3
# speedup vs baseline: 1.0068x; 1.0068x over previous
"""Causal self-attention (B=4, T=4096, D=H=1024, fp32) on 8 Trainium2 cores.

Sharding: 2 cores per batch element. Within a batch, the 32 query tiles of
128 rows are interleaved between the 2 cores (core `pair` p takes global
q-tiles p, p+2, p+4, ...), which balances the causal-attention work exactly.
Both cores run the same module; causality is enforced by per-pair mask DATA
on the last two kv-blocks of each q-tile.

K/V projection dedup: the 16 kv chunks of 256 rows are split between the
two cores of a pair. Chunks 0-1 (rows 0-511) are computed locally on BOTH
cores in bf16 (protects the numerics of short-context queries). Chunks 2-15
are pair-interleaved (core p owns chunks c >= 2 with c%2 == p); each owned
chunk is quantized to fp8-e4m3, AllGathered pair-wise through DRAM bounce
buffers, and both halves (own chunk round-trips too -- the NEFF is SPMD so
reads must be symmetric) are upconverted to bf16 into the persistent KT/V
SBUF tiles. This removes ~44%% of the K/V projection FLOPs per core.

Layout: the host stages x pre-transposed (xT, [D, T]) so no PE transposes
are needed for the Q/K/V projections. Attention scores are computed
TRANSPOSED, ST[k, q] = K @ Q^T, so that the exp'd probabilities are already
in the [k, q] layout that the P@V matmul needs as its stationary operand --
no P transposes either. Per-query softmax denominators are accumulated with
a ones-column matmul (N=1) interleaved with the PV matmuls.

Numerics: bf16 matmuls with fp32 PSUM accumulation, softmax without
max-subtraction (scores ~N(0,1) after the 1/32 scale), exp on ScalarE in
fp32 -> bf16 probabilities, final normalization in fp32. K/V rows 512+
additionally round-trip fp8-e4m3 (measured end-to-end rel err ~5e-3).
"""

import numpy as np

B, T, D, H = 4, 4096, 1024, 1024
P = 128
NCORES = 8

CK = 256          # kv chunk rows
N_DUP = 2         # chunks computed locally on both cores (rows 0..N_DUP*CK)


DEFAULT_CFG = dict(
    phases="AB",
    pa_xt_bufs=2,
    pa_psk_bufs=2, pa_psv_bufs=2,
    pa_out8_bufs=2, pa_stage_bufs=1,
    pb_xt_bufs=2, pb_qt_bufs=1,
    pb_p_bufs=4, pb_ob_bufs=2,
    pb_q_bufs=2, pb_st_bufs=3, pb_po_bufs=2,
    s_ahead=2,
    sc_desc=False,        # ascending: early q-tiles need only early kv
    warm_mms=0,
)


def _emit(ctx, tc, xqT, xkvT, wq, wk, wv, maskt, ones, outp, bins, bouts,
          T_kv, n_qt, cfg):
    import concourse.mybir as mybir

    nc = tc.nc
    f32 = mybir.dt.float32
    bf16 = mybir.dt.bfloat16
    fp8 = mybir.dt.float8e4
    Copy = mybir.ActivationFunctionType.Copy
    Exp = mybir.ActivationFunctionType.Exp
    SCALE = 1.0 / 32.0  # 1/sqrt(H)

    n_ck = T_kv // CK                 # global kv chunks
    n_own = (n_ck - N_DUP) // 2       # exchanged chunks owned per core
    n_loc = N_DUP + n_own             # chunks computed locally (xkvT cols)
    NSC = n_qt // 4                   # 512-row query superchunks
    if "A" not in cfg["phases"]:
        n_loc = 0
    if "B" not in cfg["phases"]:
        NSC = 0

    const = ctx.enter_context(tc.tile_pool(name="const", bufs=1))
    persist = ctx.enter_context(tc.tile_pool(name="persist", bufs=1))

    # maskt[:, 0:128] masks a tile's second-to-last kv block, maskt[:,128:256]
    # its last kv block ([k, q] layout). pair 0: [triu, 0]; pair 1: [1, triu].
    mask_sb = const.tile([P, 2 * P], bf16, tag="mask")
    nc.sync.dma_start(out=mask_sb, in_=maskt)
    ones_sb = const.tile([P, 1], bf16, tag="ones")
    nc.sync.dma_start(out=ones_sb, in_=ones)

    # K^T laid out [h%128, h//128, t]; V laid out [t%128, t//128, h]
    KT = persist.tile([P, 8, T_kv], bf16, tag="KT")
    V = persist.tile([P, T_kv // P, 1024], bf16, tag="V")
    # wq lives at top level so phase B needn't wait on its DMA; issued inside
    # phase A (after the first chunk's loads) so it doesn't delay wk/xt0.
    wq_sb = persist.tile([P, 8, 1024], bf16, tag="wq")
    # prefetch of the first superchunk's x^T slice, same reasoning
    xtq0 = persist.tile([P, 8, 512], bf16, tag="xtq0")

    def load_weight(wdram, wsb, eng=None):
        # DRAM [1024,1024] bf16 -> SBUF [128, 8, 1024] (d = dc*128 + p)
        eng = eng or nc.sync
        for dc in range(8):
            eng.dma_start(out=wsb[:, dc, :], in_=wdram[dc * P:(dc + 1) * P, :])

    def load_xt(xt, xsrc, c0, width, eng=None, split=False):
        # xt[:, dc, :] = xT[dc*128:(dc+1)*128, c0:c0+width]
        for dc in range(8):
            e = eng or ((nc.sync if dc % 2 else nc.gpsimd) if split else nc.gpsimd)
            e.dma_start(out=xt[:, dc, :],
                        in_=xsrc[dc * P:(dc + 1) * P, c0:c0 + width])

    from contextlib import ExitStack as _ES

    # ---------------- Phase A: K/V projection + pair exchange ----------------
    with _ES() as pa:
        wpool = pa.enter_context(tc.tile_pool(name="pa_w", bufs=1))
        xtpool = pa.enter_context(tc.tile_pool(name="pa_xt", bufs=cfg["pa_xt_bufs"]))
        out8p = pa.enter_context(
            tc.tile_pool(name="pa_out8", bufs=cfg["pa_out8_bufs"]))
        stagep = pa.enter_context(
            tc.tile_pool(name="pa_stage", bufs=cfg["pa_stage_bufs"]))
        psA_k = pa.enter_context(
            tc.tile_pool(name="pa_psk", bufs=cfg["pa_psk_bufs"], space="PSUM"))
        psA_v = pa.enter_context(
            tc.tile_pool(name="pa_psv", bufs=cfg["pa_psv_bufs"], space="PSUM"))
        wk_sb = wpool.tile([P, 8, 1024], bf16, tag="wk")
        wv_sb = wpool.tile([P, 8, 1024], bf16, tag="wv")
        load_weight(wk, wk_sb)

        if cfg["warm_mms"]:
            warm_p = pa.enter_context(
                tc.tile_pool(name="pa_warm", bufs=1, space="PSUM"))
            warm_ps = warm_p.tile([P, P], f32, tag="warm")
            for i in range(cfg["warm_mms"]):
                nc.tensor.matmul(warm_ps, lhsT=mask_sb[:, 0:P],
                                 rhs=mask_sb[:, 0:P],
                                 start=(i == 0), stop=(i == cfg["warm_mms"] - 1))

        sc0 = (NSC - 1) if cfg["sc_desc"] else 0
        for i in range(n_loc):
            t0 = i * CK
            xt = xtpool.tile([P, 8, CK], bf16, tag="xt")
            load_xt(xt, xkvT, t0, CK)
            if i == 0:
                # stream the rest of the initial working set behind wk+xt0,
                # on the otherwise-idle Activation DMA queue
                load_weight(wv, wv_sb, eng=nc.scalar)
                load_weight(wq, wq_sb, eng=nc.scalar)
                load_xt(xtq0, xqT, sc0 * 512, 512, eng=nc.sync)

            if i < N_DUP:
                # local bf16 chunk: straight into the persistent KT/V slots
                for hc in range(8):
                    kp = psA_k.tile([P, CK], f32, tag="kp")
                    for dc in range(8):
                        nc.tensor.matmul(
                            kp, lhsT=wk_sb[:, dc, hc * P:(hc + 1) * P],
                            rhs=xt[:, dc, :], start=(dc == 0), stop=(dc == 7))
                    nc.vector.tensor_copy(out=KT[:, hc, t0:t0 + CK], in_=kp)
                for tb in range(CK // P):
                    vp = psA_v.tile([P, 1024], f32, tag="vp")
                    for dc in range(8):
                        for nb in range(2):
                            nc.tensor.matmul(
                                vp[:, nb * 512:(nb + 1) * 512],
                                lhsT=xt[:, dc, tb * P:(tb + 1) * P],
                                rhs=wv_sb[:, dc, nb * 512:(nb + 1) * 512],
                                start=(dc == 0), stop=(dc == 7))
                    nc.vector.tensor_copy(out=V[:, t0 // P + tb, :], in_=vp)
            else:
                # owned exchanged chunk: quantize to fp8, AllGather with the
                # pair peer, upconvert BOTH gathered chunks into KT/V.
                u = i - N_DUP
                k8 = out8p.tile([P, 2 * CK * 8], fp8, tag="k8")  # [P, 4096]
                for hc in range(8):
                    kp = psA_k.tile([P, CK], f32, tag="kp")
                    for dc in range(8):
                        nc.tensor.matmul(
                            kp, lhsT=wk_sb[:, dc, hc * P:(hc + 1) * P],
                            rhs=xt[:, dc, :], start=(dc == 0), stop=(dc == 7))
                    nc.scalar.activation(out=k8[:, hc * CK:(hc + 1) * CK],
                                         in_=kp, func=Copy)
                for tb in range(CK // P):
                    vp = psA_v.tile([P, 1024], f32, tag="vp")
                    for dc in range(8):
                        for nb in range(2):
                            nc.tensor.matmul(
                                vp[:, nb * 512:(nb + 1) * 512],
                                lhsT=xt[:, dc, tb * P:(tb + 1) * P],
                                rhs=wv_sb[:, dc, nb * 512:(nb + 1) * 512],
                                start=(dc == 0), stop=(dc == 7))
                    nc.scalar.activation(
                        out=k8[:, 8 * CK + tb * 1024:8 * CK + (tb + 1) * 1024],
                        in_=vp, func=Copy)
                nc.scalar.dma_start(out=bins[u], in_=k8)
                nc.gpsimd.collective_compute(
                    "AllGather", mybir.AluOpType.bypass,
                    replica_groups=[[0, 1], [2, 3], [4, 5], [6, 7]],
                    ins=[bins[u]], outs=[bouts[u]])
                for half in range(2):
                    c = 2 * u + N_DUP + half
                    st = stagep.tile([P, 2 * CK * 8], fp8, tag="stage")
                    nc.sync.dma_start(
                        out=st, in_=bouts[u][half * P:(half + 1) * P, :])
                    for hc in range(8):
                        nc.vector.tensor_copy(
                            out=KT[:, hc, c * CK:(c + 1) * CK],
                            in_=st[:, hc * CK:(hc + 1) * CK])
                    for tb in range(CK // P):
                        nc.vector.tensor_copy(
                            out=V[:, (c * CK) // P + tb, :],
                            in_=st[:, 8 * CK + tb * 1024:8 * CK + (tb + 1) * 1024])

    # ---------------- Phase B: Q projection + attention ----------------
    with _ES() as pb_es:
        ec = pb_es.enter_context
        xtq_p = ec(tc.tile_pool(name="pb_xt", bufs=cfg["pb_xt_bufs"]))
        qt_p = ec(tc.tile_pool(name="pb_qt", bufs=cfg["pb_qt_bufs"]))
        pb_p = ec(tc.tile_pool(name="pb_p", bufs=cfg["pb_p_bufs"]))
        sums_p = ec(tc.tile_pool(name="pb_sums", bufs=4))
        ob_p = ec(tc.tile_pool(name="pb_ob", bufs=cfg["pb_ob_bufs"]))
        ps_st = ec(tc.tile_pool(name="pb_st", bufs=cfg["pb_st_bufs"], space="PSUM"))
        ps_o = ec(tc.tile_pool(name="pb_po", bufs=cfg["pb_po_bufs"], space="PSUM"))
        ps_sum = ec(tc.tile_pool(name="pb_psum_s", bufs=1, space="PSUM"))

        sc_order = list(range(NSC))
        if cfg["sc_desc"]:
            sc_order = sc_order[::-1]

        for si_sc, sc in enumerate(sc_order):
            # Q^T for this superchunk: [h%128, h//128, 512 local q]
            if si_sc == 0:
                xtq = xtq0
            else:
                xtq = xtq_p.tile([P, 8, 512], bf16, tag="xtq")
                load_xt(xtq, xqT, sc * 512, 512)
            qt = qt_p.tile([P, 8, 512], bf16, tag="qt")
            for hc in range(8):
                for qh in range(2):
                    qp = ps_st.tile([P, 256], f32, tag="sp", name="qp")
                    for dc in range(8):
                        nc.tensor.matmul(
                            qp, lhsT=wq_sb[:, dc, hc * P:(hc + 1) * P],
                            rhs=xtq[:, dc, qh * 256:(qh + 1) * 256],
                            start=(dc == 0), stop=(dc == 7))
                    nc.vector.tensor_copy(
                        out=qt[:, hc, qh * 256:(qh + 1) * 256], in_=qp)

            for m in range(2):
                jA = sc * 4 + 2 * m
                jB = jA + 1
                nbA, nbB = 2 * jA + 2, 2 * jB + 2  # kv blocks incl. 2 masked
                qoff = 256 * m
                # steps: (kb, width, q offset, [(tile_key, ST col offset)])
                steps = []
                for kb in range(nbA):
                    steps.append((kb, 256, qoff, [("A", 0), ("B", P)]))
                for kb in range(nbA, nbB):
                    steps.append((kb, 128, qoff + P, [("B", 0)]))
                nst = len(steps)

                op = {"A": ps_o.tile([P, 1024], f32, tag="op", name="opA"),
                      "B": ps_o.tile([P, 1024], f32, tag="op", name="opB")}
                sums = ps_sum.tile([P, 2], f32, tag="sums")
                scol = {"A": 0, "B": 1}
                lastb = {"A": nbA - 1, "B": nbB - 1}

                def s_mm(si):
                    kb, w, qo, _tiles = steps[si]
                    sp = ps_st.tile([P, w], f32, tag="sp")
                    for hc in range(8):
                        nc.tensor.matmul(
                            sp, lhsT=KT[:, hc, kb * P:(kb + 1) * P],
                            rhs=qt[:, hc, qo:qo + w],
                            start=(hc == 0), stop=(hc == 7))
                    return sp

                def softmax(si, sp):
                    kb, w, qo, tiles = steps[si]
                    pb = pb_p.tile([P, w], bf16, tag="pb")
                    nc.scalar.activation(out=pb, in_=sp, func=Exp, scale=SCALE)
                    for tk, off in tiles:
                        d = kb - (lastb[tk] - 1)   # 0: 2nd-to-last, 1: last
                        if d >= 0:
                            nc.vector.tensor_mul(
                                pb[:, off:off + P], pb[:, off:off + P],
                                mask_sb[:, d * P:(d + 1) * P])
                    return pb

                def pv(si, pb):
                    kb, w, qo, tiles = steps[si]
                    for tk, off in tiles:
                        o = op[tk]
                        first = (kb == 0)
                        last = (kb == lastb[tk])
                        for nb in range(2):
                            nc.tensor.matmul(
                                o[:, nb * 512:(nb + 1) * 512],
                                lhsT=pb[:, off:off + P],
                                rhs=V[:, kb, nb * 512:(nb + 1) * 512],
                                start=first, stop=last)
                        # sums cols A/B share one PSUM bank; start=True zeroes
                        # the WHOLE bank, so only the group's very first sums
                        # matmul (tile A, kb 0) starts, and only the group's
                        # final one (tile B's last) stops.
                        nc.tensor.matmul(
                            sums[:, scol[tk]:scol[tk] + 1],
                            lhsT=pb[:, off:off + P], rhs=ones_sb,
                            start=(first and tk == "A"),
                            stop=(last and tk == "B"),
                            skip_group_check=True)

                ahead = cfg["s_ahead"]
                sps, pbs = {}, {}
                for si in range(min(ahead, nst)):
                    sps[si] = s_mm(si)
                    pbs[si] = softmax(si, sps[si])
                for si in range(nst):
                    pv(si, pbs[si])
                    if si + ahead < nst:
                        sps[si + ahead] = s_mm(si + ahead)
                        pbs[si + ahead] = softmax(si + ahead, sps[si + ahead])

                for tk, j in (("A", jA), ("B", jB)):
                    tot = sums_p.tile([P, 1], f32, tag="tot")
                    nc.vector.tensor_copy(out=tot,
                                          in_=sums[:, scol[tk]:scol[tk] + 1])
                    rec = sums_p.tile([P, 1], f32, tag="rec")
                    nc.vector.reciprocal(out=rec, in_=tot)
                    ob = ob_p.tile([P, 1024], f32, tag="ob")
                    nc.scalar.activation(out=ob, in_=op[tk], func=Copy,
                                         scale=rec)
                    nc.sync.dma_start(out=outp[j * P:(j + 1) * P, :], in_=ob)


def build_module(T_kv=T, n_qt=None, cfg=None):
    from contextlib import ExitStack
    import concourse.tile as tile
    import concourse.mybir as mybir
    from concourse import bacc

    if n_qt is None:
        n_qt = T_kv // 256
    full_cfg = dict(DEFAULT_CFG)
    if cfg:
        full_cfg.update(cfg)
    cfg = full_cfg
    dt = mybir.dt
    nc = bacc.Bacc("TRN2", target_bir_lowering=False, debug=False,
                   num_devices=NCORES)
    n_ck = T_kv // CK
    n_own = (n_ck - N_DUP) // 2
    n_loc = N_DUP + n_own
    xqT = nc.dram_tensor("xqt", [D, n_qt * P], dt.bfloat16, kind="ExternalInput").ap()
    xkvT = nc.dram_tensor("xkvt", [D, n_loc * CK], dt.bfloat16,
                          kind="ExternalInput").ap()
    wq = nc.dram_tensor("wq", [D, H], dt.bfloat16, kind="ExternalInput").ap()
    wk = nc.dram_tensor("wk", [D, H], dt.bfloat16, kind="ExternalInput").ap()
    wv = nc.dram_tensor("wv", [D, H], dt.bfloat16, kind="ExternalInput").ap()
    maskt = nc.dram_tensor("maskt", [P, 2 * P], dt.bfloat16, kind="ExternalInput").ap()
    ones = nc.dram_tensor("ones", [P, 1], dt.bfloat16, kind="ExternalInput").ap()
    outp = nc.dram_tensor("outp", [n_qt * P, H], dt.float32, kind="ExternalOutput").ap()
    bins = [nc.dram_tensor(f"ccin{u}", [P, 2 * CK * 8], dt.float8e4,
                           kind="Internal").ap() for u in range(n_own)]
    bouts = [nc.dram_tensor(f"ccout{u}", [2 * P, 2 * CK * 8], dt.float8e4,
                            kind="Internal").ap() for u in range(n_own)]

    with tile.TileContext(nc) as tc:
        with ExitStack() as ctx:
            _emit(ctx, tc, xqT, xkvT, wq, wk, wv, maskt, ones, outp,
                  bins, bouts, T_kv, n_qt, cfg)
    nc.compile()
    return nc


def host_inputs(x, Wq, Wk, Wv, T_kv=T, n_qt=None, n_batch=None):
    """Build the per-core input maps for run_bass_kernel_spmd."""
    import ml_dtypes
    bf = ml_dtypes.bfloat16
    if n_qt is None:
        n_qt = T_kv // 256
    if n_batch is None:
        n_batch = x.shape[0]
    n_ck = T_kv // CK
    n_own = (n_ck - N_DUP) // 2
    triu = np.triu(np.ones((P, P), np.float32))   # mask[k,q] = 1 iff k<=q
    m = [np.concatenate([triu, np.zeros((P, P), np.float32)], 1).astype(bf),
         np.concatenate([np.ones((P, P), np.float32), triu], 1).astype(bf)]
    onesv = np.ones((P, 1), np.float32).astype(bf)

    xb = np.asarray(x, np.float32).astype(bf)
    wqb = np.asarray(Wq, np.float32).astype(bf)
    wkb = np.asarray(Wk, np.float32).astype(bf)
    wvb = np.asarray(Wv, np.float32).astype(bf)
    in_maps = []
    for c in range(NCORES):
        b, pair = (c // 2) % n_batch, c % 2
        xT = np.ascontiguousarray(xb[b].T)            # [D, T]
        qcols = np.concatenate(
            [xT[:, (2 * j + pair) * P:(2 * j + pair + 1) * P]
             for j in range(n_qt)], axis=1)
        kvcols = [xT[:, 0:N_DUP * CK]] + [
            xT[:, (2 * u + N_DUP + pair) * CK:(2 * u + N_DUP + pair + 1) * CK]
            for u in range(n_own)]
        in_maps.append({
            "xqt": np.ascontiguousarray(qcols),
            "xkvt": np.ascontiguousarray(np.concatenate(kvcols, axis=1)),
            "wq": wqb, "wk": wkb, "wv": wvb,
            "maskt": m[pair], "ones": onesv,
        })
    return in_maps


def gather_output(results, T_kv=T, n_qt=None, n_batch=B):
    if n_qt is None:
        n_qt = T_kv // 256
    out = np.empty((n_batch, T_kv, H), np.float32)
    for c in range(2 * n_batch):
        b, pair = c // 2, c % 2
        r = results[c]["outp"]
        for j in range(n_qt):
            out[b, (2 * j + pair) * P:(2 * j + pair + 1) * P, :] = \
                r[j * P:(j + 1) * P, :]
    return out


_NC_CACHE = {}


def kernel(x, Wq, Wk, Wv):
    from concourse.bass_utils import run_bass_kernel_spmd

    x = np.asarray(x, dtype=np.float32)
    Wq = np.asarray(Wq, dtype=np.float32)
    Wk = np.asarray(Wk, dtype=np.float32)
    Wv = np.asarray(Wv, dtype=np.float32)

    if "nc" not in _NC_CACHE:
        _NC_CACHE["nc"] = build_module()
    nc = _NC_CACHE["nc"]

    in_maps = host_inputs(x, Wq, Wk, Wv)
    res = run_bass_kernel_spmd(nc, in_maps, core_ids=list(range(NCORES)))
    return gather_output(res.results)
